# revision 1
# baseline (speedup 1.0000x reference)
"""GCN layer on 8 Trainium2 NeuronCores.

out = D^-1/2 A D^-1/2 (values @ W + b),  A: [8192, 8192] f32 dense.

Strategy (row-parallel, single pass over A):
- Shard A row-wise: core k gets rows [k*1024, (k+1)*1024).
- Stream the fp32 slab once; PE-transpose 128x128 tiles (fp32 transpose mode),
  copy-cast PSUM->SBUF to a bf16 transposed cache ATC [j-part, i-free] (16MB).
- Row sums d via matmul(ones, ATC) accumulated in PSUM -> AllGather d (4KB).
- dis = rsqrt(d) (ACT Rsqrt + one Newton step).
- Y = (values @ W + b) * dis_j computed in-place on a bf16 fc buffer
  (values^T passed pre-transposed from host; contraction runs on-device).
- Main matmul (Form B): out^T[o, i] += Y[jt]^T @ ATC[jt] over 64 j-tiles,
  scale by dis_i via partition-broadcast row, DMA out^T; host transposes back.
"""
import os
import numpy as np

N, D, OUT = 8192, 128, 128
N_CORES = 8
ROWS = N // N_CORES          # 1024 rows of A per core
NJT = N // 128               # 64 j-tiles
NIT = ROWS // 128            # 8 i-blocks
JC = 2048                    # staged j-chunk width (fp32)
NJC = N // JC                # 4 chunks
NG = JC // 512               # 4 transpose groups per stage tile

_CACHE = {}


def _inv_sqrt(nc, mybir, pool, d_ap, shape):
    """dis = 1/(sqrt(d) + 1e-8) via ACT Sqrt + DVE reciprocal."""
    F32 = mybir.dt.float32
    s = pool.tile(list(shape), F32, tag="nsq")
    nc.scalar.activation(s[:], d_ap, mybir.ActivationFunctionType.Sqrt)
    nc.vector.tensor_scalar_add(s[:], s[:], 1e-8)
    dis = pool.tile(list(shape), F32, tag="ndis")
    nc.vector.reciprocal(dis[:], s[:])
    return dis


def _build():
    import concourse.bacc as bacc
    import concourse.mybir as mybir
    import concourse.tile as tile

    F32, BF16 = mybir.dt.float32, mybir.dt.bfloat16
    nc = bacc.Bacc(None, target_bir_lowering=False, num_devices=N_CORES)

    a_in = nc.declare_dram_parameter("a", [ROWS, N], F32, isOutput=False)
    vt_in = nc.declare_dram_parameter("vt", [D, N], F32, isOutput=False)
    w_in = nc.declare_dram_parameter("w", [D, OUT], F32, isOutput=False)
    bb_in = nc.declare_dram_parameter("bb", [128, OUT], F32, isOutput=False)
    id_in = nc.declare_dram_parameter("ident", [128, 128], F32, isOutput=False)
    outT = nc.declare_dram_parameter("outT", [OUT, ROWS], F32, isOutput=True)

    with tile.TileContext(nc) as tc:
        with (
            tc.tile_pool(name="const", bufs=1) as constp,
            tc.tile_pool(name="stage", bufs=2) as stage,
            tc.tile_pool(name="small", bufs=1) as small,
            tc.tile_pool(name="pst", bufs=3, space="PSUM") as pst,
            tc.tile_pool(name="psa", bufs=2, space="PSUM") as psa,
            tc.tile_pool(name="psd", bufs=1, space="PSUM") as psd,
            tc.tile_pool(name="dram", bufs=1, space="DRAM") as dram,
        ):
            # constants
            ident = constp.tile([128, 128], F32)
            nc.sync.dma_start(out=ident[:], in_=id_in[:])
            w_sb = constp.tile([D, OUT], F32)
            nc.sync.dma_start(out=w_sb[:], in_=w_in[:])
            w_bf = constp.tile([D, OUT], BF16)
            nc.vector.tensor_copy(w_bf[:], w_sb[:])
            bb_sb = constp.tile([128, OUT], F32)
            nc.sync.dma_start(out=bb_sb[:], in_=bb_in[:])
            ones_bf = constp.tile([128, 1], BF16)
            nc.vector.memset(ones_bf[:], 1.0)

            # big caches
            ATC = constp.tile([128, NJT * 1024], BF16)   # 16MB transposed A (bf16)
            fcY = constp.tile([128, NJT * 128], BF16)    # 2MB fc_sc, then Y in place
            vt_bf = constp.tile([D, N], BF16)            # 2MB values^T bf16

            # values^T: stage fp32 chunks, cast to bf16
            for c in range(NJC):
                vstg = stage.tile([128, JC], F32, tag="stg")
                nc.sync.dma_start(out=vstg[:], in_=vt_in[:, c * JC : (c + 1) * JC])
                nc.vector.tensor_copy(vt_bf[:, c * JC : (c + 1) * JC], vstg[:])

            # fc = values @ W + b  -> fcY (bf16), tile nt covers rows nt*128..
            for nt in range(NJT):
                fc_ps = psa.tile([128, OUT], F32, tag="acc")
                nc.tensor.matmul(
                    fc_ps[:], vt_bf[:, nt * 128 : (nt + 1) * 128], w_bf[:],
                    start=True, stop=True,
                )
                nc.vector.tensor_tensor(
                    out=fcY[:, nt * 128 : (nt + 1) * 128],
                    in0=fc_ps[:], in1=bb_sb[:], op=mybir.AluOpType.add,
                )

            # d accumulators (persist across the stream)
            d_ps = [psd.tile([1, 512], F32, tag=f"d{h}", name=f"dps{h}") for h in range(2)]

            ATC3 = ATC[:].rearrange("p (j i) -> p j i", j=NJT)

            # stream A: chunk-major over j so d-matmuls fire per chunk wave
            for jc in range(NJC):
                for it in range(NIT):
                    st = stage.tile([128, JC], F32, tag="stg")
                    nc.sync.dma_start(
                        out=st[:],
                        in_=a_in[it * 128 : (it + 1) * 128, jc * JC : (jc + 1) * JC],
                    )
                    for g in range(NG):
                        ps = pst.tile([128, 512], F32, tag="tp")
                        for m in range(4):
                            # one accumulation group per PSUM tile: only the
                            # first write clears the bank's has_written bits
                            nc.tensor.matmul(
                                ps[:, m * 128 : (m + 1) * 128],
                                st[:, (g * 4 + m) * 128 : (g * 4 + m + 1) * 128],
                                ident[:],
                                is_transpose=True,
                                start=(m == 0), stop=(m == 3),
                            )
                        jt0 = jc * (JC // 128) + g * 4
                        nc.vector.tensor_copy(
                            ATC3[:, jt0 : jt0 + 4, it * 128 : (it + 1) * 128],
                            ps[:].rearrange("p (m i) -> p m i", m=4),
                        )
                # row-sum matmuls for the 16 j-tiles completed in this chunk
                for jt in range(jc * (JC // 128), (jc + 1) * (JC // 128)):
                    for h in range(2):
                        nc.tensor.matmul(
                            d_ps[h][:], ones_bf[:],
                            ATC[:, jt * 1024 + h * 512 : jt * 1024 + (h + 1) * 512],
                            start=(jt == 0), stop=(jt == NJT - 1),
                        )

            # local d -> DRAM -> AllGather(8 cores) -> full d
            d_row = small.tile([1, ROWS], F32)
            for h in range(2):
                nc.vector.tensor_copy(d_row[0:1, h * 512 : (h + 1) * 512], d_ps[h][:])
            d_loc = dram.tile([ROWS], F32)
            d_full = dram.tile([N], F32, addr_space="Shared")
            nc.sync.dma_start(out=d_loc[:], in_=d_row[:])
            nc.gpsimd.collective_compute(
                "AllGather", mybir.AluOpType.bypass,
                replica_groups=[list(range(N_CORES))],
                ins=[d_loc[:].opt()], outs=[d_full[:].opt()],
            )

            # full d as [128, 64] columns (partition = within-tile row index)
            d_cols = small.tile([128, NJT], F32)
            for t in range(NJT):
                nc.sync.dma_start(
                    out=d_cols[:, t : t + 1],
                    in_=d_full[t * 128 : (t + 1) * 128].rearrange("(p o) -> p o", o=1),
                )
            dis_cols = _inv_sqrt(nc, mybir, small, d_cols[:], (128, NJT))
            # local dis row for the output row scale (uses local d, no core offset)
            dis_row = _inv_sqrt(nc, mybir, small, d_row[:], (1, ROWS))

            # Y = fc * dis_j  (in place, bf16)
            for jt in range(NJT):
                nc.vector.tensor_scalar(
                    out=fcY[:, jt * 128 : (jt + 1) * 128],
                    in0=fcY[:, jt * 128 : (jt + 1) * 128],
                    scalar1=dis_cols[:, jt : jt + 1], scalar2=None,
                    op0=mybir.AluOpType.mult,
                )

            # main matmul: outT[o, i] = sum_jt Y[jt]^T @ ATC[jt]
            oT = [psa.tile([128, 512], F32, tag="acc", name=f"oT{h}") for h in range(2)]
            for jt in range(NJT):
                for h in range(2):
                    nc.tensor.matmul(
                        oT[h][:], fcY[:, jt * 128 : (jt + 1) * 128],
                        ATC[:, jt * 1024 + h * 512 : jt * 1024 + (h + 1) * 512],
                        start=(jt == 0), stop=(jt == NJT - 1),
                    )
            # epilogue: scale by dis_i along the free axis. Broadcast dis_row
            # across partitions via a K=1 outer-product matmul, then multiply.
            ones_row = constp.tile([1, 128], F32)
            nc.vector.memset(ones_row[:], 1.0)
            for h in range(2):
                bc_ps = pst.tile([128, 512], F32, tag="tp")
                nc.tensor.matmul(
                    bc_ps[:], ones_row[:], dis_row[0:1, h * 512 : (h + 1) * 512],
                    start=True, stop=True,
                )
                dis_bc = stage.tile([128, 512], F32, tag="dbc")
                nc.vector.tensor_copy(dis_bc[:], bc_ps[:])
                osb = stage.tile([128, 512], F32, tag="osb")
                nc.vector.tensor_tensor(
                    out=osb[:], in0=oT[h][:], in1=dis_bc[:],
                    op=mybir.AluOpType.mult,
                )
                nc.sync.dma_start(out=outT[:, h * 512 : (h + 1) * 512], in_=osb[:])

    nc.compile()
    return nc


def kernel(values, adjacency, W, b):
    from concourse.bass_utils import run_bass_kernel_spmd

    if "nc" not in _CACHE:
        _CACHE["nc"] = _build()
    nc = _CACHE["nc"]

    values = np.asarray(values, dtype=np.float32)
    adjacency = np.ascontiguousarray(np.asarray(adjacency, dtype=np.float32))
    W = np.asarray(W, dtype=np.float32)
    b = np.asarray(b, dtype=np.float32)

    vt = np.ascontiguousarray(values.T)                  # [D, N]
    bb = np.ascontiguousarray(np.tile(b[None, :], (128, 1)))
    ident = np.eye(128, dtype=np.float32)

    in_maps = [
        {
            "a": adjacency[k * ROWS : (k + 1) * ROWS],
            "vt": vt, "w": W, "bb": bb, "ident": ident,
        }
        for k in range(N_CORES)
    ]
    trace = bool(int(os.environ.get("GCN_TRACE", "0")))
    res = run_bass_kernel_spmd(nc, in_maps, list(range(N_CORES)), trace=trace)
    if trace and res.exec_time_ns is not None:
        print(f"HW exec time: {res.exec_time_ns} ns")
        _CACHE["exec_time_ns"] = res.exec_time_ns
    out = np.concatenate(
        [res.results[k]["outT"].T for k in range(N_CORES)], axis=0
    ).astype(np.float32)
    return out



# revision 5
# speedup vs baseline: 1.4424x; 1.4424x over previous
"""GCN layer on 8 Trainium2 NeuronCores.

out = D^-1/2 A D^-1/2 (values @ W + b),  A: [8192, 8192] f32 dense.

Strategy (row-parallel, host-transposed slabs, split-gather overlap):
- Core k owns output rows Rk = [1024k, 1024(k+1)). Host passes the slab
  pre-transposed: at = A[Rk, :].T  -> [8192 j, 1024 i], so tiles DMA with
  the contraction dim j on partitions (no on-device PE transposes).
- Stream at in two i-phases (cols 0:512, then 512:1024). Each 1MB stage
  DMA carries 4 j-tiles; DVE casts fp32->bf16 into a resident 16MB cache
  ATC [j-part, jt*1024 + i]. Row sums d (ones^T @ tile) accumulate in
  PSUM per phase, overlapping the stream.
- After phase A: dis_a = 1/(sqrt(d_a)+1e-8) locally -> AllGather-1 runs
  while phase B streams. After phase B: AllGather-2 runs while the 32
  S1 j-tile main matmuls execute.
- dis distribution: one contiguous DMA [32,128] + one PE transpose.
- Y = fc * dis_j in place (bf16); main matmul out^T[o,i] += Y_jt^T @
  ATC_jt; epilogue scales by dis_i (partition-broadcast via K=1 matmul);
  host transposes out^T back.
"""
import os
import numpy as np

N, D, OUT = 8192, 128, 128
N_CORES = 8
ROWS = N // N_CORES          # 1024 rows of A per core
NJT = N // 128               # 64 j-tiles
NST = 16                     # stages per phase (4 j-tiles each)
HALF = 512                   # i-split width per phase

_CACHE = {}


def _build():
    import concourse.bacc as bacc
    import concourse.mybir as mybir
    import concourse.tile as tile

    F32, BF16 = mybir.dt.float32, mybir.dt.bfloat16
    nc = bacc.Bacc(None, target_bir_lowering=False, num_devices=N_CORES)

    at_in = nc.declare_dram_parameter("at", [N, ROWS], F32, isOutput=False)
    vt_in = nc.declare_dram_parameter("vt", [D, N], F32, isOutput=False)
    w_in = nc.declare_dram_parameter("w", [D, OUT], F32, isOutput=False)
    bb_in = nc.declare_dram_parameter("bb", [128, OUT], F32, isOutput=False)
    id_in = nc.declare_dram_parameter("ident", [128, 128], F32, isOutput=False)
    outT = nc.declare_dram_parameter("outT", [OUT, ROWS], F32, isOutput=True)

    def inv_sqrt_to(pool, dst_ap, src_ap, shape):
        s = pool.tile(list(shape), F32, tag="nsq")
        nc.scalar.activation(s[:], src_ap, mybir.ActivationFunctionType.Sqrt)
        nc.vector.tensor_scalar_add(s[:], s[:], 1e-8)
        nc.vector.reciprocal(dst_ap, s[:])

    with tile.TileContext(nc) as tc:
        with (
            tc.tile_pool(name="const", bufs=1) as constp,
            tc.tile_pool(name="stage", bufs=4) as stage,
            tc.tile_pool(name="epi", bufs=2) as epip,
            tc.tile_pool(name="vtb", bufs=2) as vtbp,
            tc.tile_pool(name="small", bufs=1) as small,
            tc.tile_pool(name="ps", bufs=2, space="PSUM") as ps,
            tc.tile_pool(name="po", bufs=1, space="PSUM") as po,
            tc.tile_pool(name="pd", bufs=1, space="PSUM") as pd,
            tc.tile_pool(name="dram", bufs=1, space="DRAM") as dram,
        ):
            # ---- constants ----
            ident = constp.tile([128, 128], F32)
            nc.sync.dma_start(out=ident[:], in_=id_in[:])
            w_sb = constp.tile([D, OUT], F32)
            nc.sync.dma_start(out=w_sb[:], in_=w_in[:])
            w_bf = constp.tile([D, OUT], BF16)
            nc.vector.tensor_copy(w_bf[:], w_sb[:])
            bb_sb = constp.tile([128, OUT], F32)
            nc.sync.dma_start(out=bb_sb[:], in_=bb_in[:])
            ones_bf = constp.tile([128, 1], BF16)
            nc.vector.memset(ones_bf[:], 1.0)
            ones_row = constp.tile([1, 128], F32)
            nc.vector.memset(ones_row[:], 1.0)
            Z = constp.tile([128, 128], F32)
            nc.vector.memset(Z[:], 0.0)

            # ---- big persistent buffers ----
            ATC = constp.tile([128, NJT * 1024], BF16)   # 16MB transposed A (bf16)
            fcY = constp.tile([128, NJT * 128], BF16)    # 2MB fc_sc, then Y in place
            dis_cols = constp.tile([128, 64], F32)       # dis_j per tile column
            dis_row = constp.tile([1, ROWS], F32)        # local dis_i row

            ATC3 = ATC[:].rearrange("p (j i) -> p j i", j=NJT)

            # ---- fc = values @ W + b  (bf16 into fcY) ----
            for c in range(4):
                vstg = stage.tile([128, 2048], F32, tag="stg")
                nc.sync.dma_start(out=vstg[:], in_=vt_in[:, c * 2048 : (c + 1) * 2048])
                vb = vtbp.tile([128, 2048], BF16, tag="vtb")
                nc.vector.tensor_copy(vb[:], vstg[:])
                for m in range(16):
                    nt = c * 16 + m
                    fc_ps = ps.tile([128, OUT], F32, tag="fc")
                    nc.tensor.matmul(
                        fc_ps[:], vb[:, m * 128 : (m + 1) * 128], w_bf[:],
                        start=True, stop=True,
                    )
                    nc.vector.tensor_tensor(
                        out=fcY[:, nt * 128 : (nt + 1) * 128],
                        in0=fc_ps[:], in1=bb_sb[:], op=mybir.AluOpType.add,
                    )

            # ---- degree accumulators ----
            d_ps = [pd.tile([1, HALF], F32, tag=f"d{h}", name=f"dps{h}") for h in range(2)]

            # ---- stream phases: ph 0 = i cols [0,512), ph 1 = [512,1024) ----
            dis_loc = [None, None]
            dis_full = [None, None]
            for ph in range(2):
                for s in range(NST):
                    st = stage.tile([128, 2048], F32, tag="stg")
                    nc.sync.dma_start(
                        out=st[:].rearrange("p (q c) -> p q c", q=4),
                        in_=at_in[
                            s * 512 : (s + 1) * 512, ph * HALF : (ph + 1) * HALF
                        ].rearrange("(q p) c -> p q c", p=128),
                    )
                    nc.vector.tensor_copy(
                        ATC3[:, 4 * s : 4 * s + 4, ph * HALF : (ph + 1) * HALF],
                        st[:].rearrange("p (q c) -> p q c", q=4),
                    )
                    for q in range(4):
                        jt = 4 * s + q
                        nc.tensor.matmul(
                            d_ps[ph][:], ones_bf[:],
                            ATC[:, jt * 1024 + ph * HALF : jt * 1024 + (ph + 1) * HALF],
                            start=(s == 0 and q == 0), stop=(s == NST - 1 and q == 3),
                        )
                # local dis for this phase's rows -> DRAM -> AllGather
                inv_sqrt_to(
                    small, dis_row[0:1, ph * HALF : (ph + 1) * HALF], d_ps[ph][:],
                    (1, HALF),
                )
                dis_loc[ph] = dram.tile([HALF], F32, name=f"disloc{ph}")
                dis_full[ph] = dram.tile(
                    [HALF * N_CORES], F32, addr_space="Shared", name=f"disfull{ph}"
                )
                nc.sync.dma_start(
                    out=dis_loc[ph][:], in_=dis_row[0:1, ph * HALF : (ph + 1) * HALF]
                )
                nc.gpsimd.collective_compute(
                    "AllGather", mybir.AluOpType.bypass,
                    replica_groups=[list(range(N_CORES))],
                    ins=[dis_loc[ph][:].opt()], outs=[dis_full[ph][:].opt()],
                )

            # ---- per-half: distribute dis, scale Y, run main matmuls ----
            oT = [po.tile([128, HALF], F32, tag=f"o{h}", name=f"oT{h}") for h in range(2)]
            # tiles whose dis arrives in gather g: jt%8 in [4g, 4g+4)
            sets = [
                [jt for jt in range(NJT) if (jt % 8) // 4 == g] for g in range(2)
            ]
            for g in range(2):
                # dis_full[g] -> Z rows -> PE transpose -> dis_cols[:, 32g:32g+32]
                nc.sync.dma_start(
                    out=Z[0:32, :],
                    in_=dis_full[g][:].rearrange("(t p) -> t p", p=128),
                )
                zt_ps = ps.tile([128, 128], F32, tag="fc")
                nc.tensor.matmul(zt_ps[:], Z[:], ident[:], is_transpose=True,
                                 start=True, stop=True)
                nc.vector.tensor_copy(
                    dis_cols[:, 32 * g : 32 * g + 32], zt_ps[:, 0:32]
                )
                # Y = fc * dis_j for this gather's tiles
                for jt in sets[g]:
                    t = 32 * g + 4 * (jt // 8) + (jt % 8) - 4 * g
                    nc.vector.tensor_scalar(
                        out=fcY[:, jt * 128 : (jt + 1) * 128],
                        in0=fcY[:, jt * 128 : (jt + 1) * 128],
                        scalar1=dis_cols[:, t : t + 1], scalar2=None,
                        op0=mybir.AluOpType.mult,
                    )
                # main matmuls for this gather's tiles (both i-halves)
                for jt in sets[g]:
                    for h in range(2):
                        nc.tensor.matmul(
                            oT[h][:], fcY[:, jt * 128 : (jt + 1) * 128],
                            ATC[:, jt * 1024 + h * HALF : jt * 1024 + (h + 1) * HALF],
                            start=(jt == sets[0][0]), stop=(jt == sets[1][-1]),
                        )

            # ---- epilogue: scale by dis_i (broadcast over partitions), DMA out ----
            for h in range(2):
                bc_ps = ps.tile([128, HALF], F32, tag="bc")
                nc.tensor.matmul(
                    bc_ps[:], ones_row[:], dis_row[0:1, h * HALF : (h + 1) * HALF],
                    start=True, stop=True,
                )
                dbc = epip.tile([128, HALF], F32, tag="dbc")
                nc.vector.tensor_copy(dbc[:], bc_ps[:])
                osb = epip.tile([128, HALF], F32, tag="osb")
                nc.vector.tensor_tensor(
                    out=osb[:], in0=oT[h][:], in1=dbc[:], op=mybir.AluOpType.mult,
                )
                nc.sync.dma_start(out=outT[:, h * HALF : (h + 1) * HALF], in_=osb[:])

    nc.compile()
    return nc


def kernel(values, adjacency, W, b):
    from concourse.bass_utils import run_bass_kernel_spmd

    if "nc" not in _CACHE:
        _CACHE["nc"] = _build()
    nc = _CACHE["nc"]

    values = np.asarray(values, dtype=np.float32)
    adjacency = np.asarray(adjacency, dtype=np.float32)
    W = np.asarray(W, dtype=np.float32)
    b = np.asarray(b, dtype=np.float32)

    vt = np.ascontiguousarray(values.T)                  # [D, N]
    bb = np.ascontiguousarray(np.tile(b[None, :], (128, 1)))
    ident = np.eye(128, dtype=np.float32)

    in_maps = [
        {
            "at": np.ascontiguousarray(adjacency[k * ROWS : (k + 1) * ROWS].T),
            "vt": vt, "w": W, "bb": bb, "ident": ident,
        }
        for k in range(N_CORES)
    ]
    trace = bool(int(os.environ.get("GCN_TRACE", "0")))
    res = run_bass_kernel_spmd(nc, in_maps, list(range(N_CORES)), trace=trace)
    if trace and res.exec_time_ns is not None:
        print(f"HW exec time: {res.exec_time_ns} ns")
        _CACHE["exec_time_ns"] = res.exec_time_ns
    out = np.concatenate(
        [res.results[k]["outT"].T for k in range(N_CORES)], axis=0
    ).astype(np.float32)
    return out


# revision 14
# speedup vs baseline: 1.6110x; 1.1169x over previous
"""GCN layer on 8 Trainium2 NeuronCores.

out = D^-1/2 A D^-1/2 (values @ W + b),  A: [8192, 8192] f32 dense.

Strategy (row-parallel, host-transposed slabs, split-gather overlap):
- Core k owns output rows Rk = [1024k, 1024(k+1)). Host passes the slab
  pre-transposed: at = A[Rk, :].T  -> [8192 j, 1024 i], so tiles DMA with
  the contraction dim j on partitions (no on-device PE transposes).
- Stream at in two i-phases (cols 0:512, then 512:1024). Each 512KB stage
  DMA carries 2 j-tiles; DVE casts fp32->bf16 into a resident 16MB cache
  ATC [j-part, jt*1024 + i]. Row sums d (ones^T @ tile) accumulate in
  two ping-pong PSUM banks per phase, overlapping the stream.
- After phase A: dis_a = Rsqrt(d_a) locally -> AllGather-1 runs while
  phase B streams (values^T + fc also stream/compute in phase B).
  After phase B: AllGather-2 runs while the 32 S1 j-tile main matmuls
  execute. A tiny warm-up AllGather at kernel start absorbs the CC
  engine's first-collective init cost.
- dis distribution: one contiguous DMA [32,128] + one PE transpose.
- Y = fc * dis_j in place (bf16); main matmul out^T[o,i] += Y_jt^T @
  ATC_jt; epilogue scales by dis_i (partition-broadcast via K=1 matmul,
  precomputed per half as soon as local dis is ready); host transposes
  out^T back.
"""
import os
import numpy as np

N, D, OUT = 8192, 128, 128
N_CORES = 8
ROWS = N // N_CORES          # 1024 rows of A per core
NJT = N // 128               # 64 j-tiles
NST = 32                     # stages per phase (2 j-tiles each)
HALF = 512                   # i-split width per phase

_CACHE = {}


def _build():
    import concourse.bacc as bacc
    import concourse.mybir as mybir
    import concourse.tile as tile

    F32, BF16 = mybir.dt.float32, mybir.dt.bfloat16
    nc = bacc.Bacc(None, target_bir_lowering=False, num_devices=N_CORES)

    at_in = nc.declare_dram_parameter("at", [N, ROWS], F32, isOutput=False)
    vt_in = nc.declare_dram_parameter("vt", [D, N], F32, isOutput=False)
    w_in = nc.declare_dram_parameter("w", [D, OUT], F32, isOutput=False)
    bb_in = nc.declare_dram_parameter("bb", [128, OUT], F32, isOutput=False)
    id_in = nc.declare_dram_parameter("ident", [128, 128], F32, isOutput=False)
    outT = nc.declare_dram_parameter("outT", [OUT, ROWS], F32, isOutput=True)

    with tile.TileContext(nc) as tc:
        with (
            tc.tile_pool(name="const", bufs=1) as constp,
            tc.tile_pool(name="stage", bufs=8) as stage,
            tc.tile_pool(name="epi", bufs=2) as epip,
            tc.tile_pool(name="vtb", bufs=2) as vtbp,
            tc.tile_pool(name="small", bufs=2) as small,
            tc.tile_pool(name="ps", bufs=2, space="PSUM") as ps,
            tc.tile_pool(name="po", bufs=1, space="PSUM") as po,
            tc.tile_pool(name="pd", bufs=1, space="PSUM") as pd,
            tc.tile_pool(name="pb", bufs=1, space="PSUM") as pb,
            tc.tile_pool(name="dram", bufs=1, space="DRAM") as dram,
        ):
            # ---- constants ----
            ident = constp.tile([128, 128], F32)
            nc.sync.dma_start(out=ident[:], in_=id_in[:])
            w_sb = constp.tile([D, OUT], F32)
            nc.sync.dma_start(out=w_sb[:], in_=w_in[:])
            w_bf = constp.tile([D, OUT], BF16)
            nc.vector.tensor_copy(w_bf[:], w_sb[:])
            bb_sb = constp.tile([128, OUT], F32)
            nc.sync.dma_start(out=bb_sb[:], in_=bb_in[:])
            ones_bf = constp.tile([128, 1], BF16)
            nc.vector.memset(ones_bf[:], 1.0)
            ones_row = constp.tile([1, 128], F32)
            nc.vector.memset(ones_row[:], 1.0)
            Z = constp.tile([128, 128], F32)
            nc.vector.memset(Z[:], 0.0)

            # warm-up collective: absorbs CC mesh-init + launch skew early,
            # while the stream is DMA-bound and the CC engine is idle.
            wu_loc = dram.tile([8], F32, name="wuloc")
            wu_full = dram.tile([8 * N_CORES], F32, addr_space="Shared", name="wufull")
            nc.sync.dma_start(out=wu_loc[:], in_=ones_row[0:1, 0:8])
            nc.gpsimd.collective_compute(
                "AllGather", mybir.AluOpType.bypass,
                replica_groups=[list(range(N_CORES))],
                ins=[wu_loc[:].opt()], outs=[wu_full[:].opt()],
            )

            # ---- big persistent buffers ----
            ATC = constp.tile([128, NJT * 1024], BF16)   # 16MB transposed A (bf16)
            fcY = constp.tile([128, NJT * 128], BF16)    # 2MB fc_sc, then Y in place
            dis_cols = constp.tile([128, 64], F32)       # dis_j per tile column
            dis_row = constp.tile([1, ROWS], F32)        # local dis_i row

            ATC3 = ATC[:].rearrange("p (j i) -> p j i", j=NJT)

            # ---- degree ping-pong accumulators (reused across phases) ----
            d_pp = [pd.tile([1, HALF], F32, tag=f"d{x}", name=f"dpp{x}") for x in range(2)]
            oT = [po.tile([128, HALF], F32, tag=f"o{h}", name=f"oT{h}") for h in range(2)]
            bc_ps = [pb.tile([128, HALF], F32, tag=f"b{h}", name=f"bc{h}") for h in range(2)]
            dbc = [None, None]
            dis_loc = [None, None]
            dis_full = [None, None]
            srows = [None, None]

            # ---- stream phases: ph 0 = i cols [0,512), ph 1 = [512,1024) ----
            for ph in range(2):
                if ph == 1:
                    # fc = values @ W + b: streamed in phase B where DMA has slack
                    for c in range(8):
                        vstg = stage.tile([128, 1024], F32, tag="stg")
                        nc.sync.dma_start(
                            out=vstg[:], in_=vt_in[:, c * 1024 : (c + 1) * 1024]
                        )
                        vb = vtbp.tile([128, 1024], BF16, tag="vtb")
                        nc.vector.tensor_copy(vb[:], vstg[:])
                        for m in range(8):
                            nt = c * 8 + m
                            fc_ps = ps.tile([128, OUT], F32, tag="fc")
                            nc.tensor.matmul(
                                fc_ps[:], vb[:, m * 128 : (m + 1) * 128], w_bf[:],
                                start=True, stop=True,
                            )
                            nc.vector.tensor_tensor(
                                out=fcY[:, nt * 128 : (nt + 1) * 128],
                                in0=fc_ps[:], in1=bb_sb[:], op=mybir.AluOpType.add,
                            )
                for s in range(NST):
                    st = stage.tile([128, 1024], F32, tag="stg")
                    nc.sync.dma_start(
                        out=st[:].rearrange("p (q c) -> p q c", q=2),
                        in_=at_in[
                            s * 256 : (s + 1) * 256, ph * HALF : (ph + 1) * HALF
                        ].rearrange("(q p) c -> p q c", p=128),
                    )
                    nc.vector.tensor_copy(
                        ATC3[:, 2 * s : 2 * s + 2, ph * HALF : (ph + 1) * HALF],
                        st[:].rearrange("p (q c) -> p q c", q=2),
                    )
                    for q in range(2):
                        jt = 2 * s + q
                        nc.tensor.matmul(
                            d_pp[q][:], ones_bf[:],
                            ATC[:, jt * 1024 + ph * HALF : jt * 1024 + (ph + 1) * HALF],
                            start=(s == 0), stop=(s == NST - 1),
                        )
                # local s = sqrt(d) + eps for this phase's rows; gather s (the
                # cheap reciprocal happens post-gather on a wide layout)
                dsum = small.tile([1, HALF], F32, tag="dsum")
                nc.vector.tensor_copy(dsum[:], d_pp[0][:])
                nc.vector.tensor_tensor(
                    out=dsum[:], in0=dsum[:], in1=d_pp[1][:], op=mybir.AluOpType.add
                )
                srow = small.tile([1, HALF], F32, tag=f"srow{ph}")
                nc.scalar.activation(
                    srow[:], dsum[:], mybir.ActivationFunctionType.Sqrt
                )
                nc.vector.tensor_scalar_add(srow[:], srow[:], 1e-8)
                dis_loc[ph] = dram.tile([HALF], F32, name=f"disloc{ph}")
                dis_full[ph] = dram.tile(
                    [HALF * N_CORES], F32, addr_space="Shared", name=f"disfull{ph}"
                )
                nc.sync.dma_start(out=dis_loc[ph][:], in_=srow[:])
                nc.gpsimd.collective_compute(
                    "AllGather", mybir.AluOpType.bypass,
                    replica_groups=[list(range(N_CORES))],
                    ins=[dis_loc[ph][:].opt()], outs=[dis_full[ph][:].opt()],
                )
                srows[ph] = srow

            # ---- per-gather: distribute dis, scale Y, run main matmuls ----
            # tiles whose dis arrives in gather g: jt%8 in [4g, 4g+4)
            sets = [
                [jt for jt in range(NJT) if (jt % 8) // 4 == g] for g in range(2)
            ]
            for g in range(2):
                # dis_full[g] -> Z rows -> PE transpose -> dis_cols[:, 32g:32g+32]
                nc.sync.dma_start(
                    out=Z[0:32, :],
                    in_=dis_full[g][:].rearrange("(t p) -> t p", p=128),
                )
                nc.vector.reciprocal(Z[0:32, :], Z[0:32, :])
                zt_ps = ps.tile([128, 128], F32, tag="fc")
                nc.tensor.matmul(zt_ps[:], Z[:], ident[:], is_transpose=True,
                                 start=True, stop=True)
                nc.vector.tensor_copy(
                    dis_cols[:, 32 * g : 32 * g + 32], zt_ps[:, 0:32]
                )
                # Y = fc * dis_j, then main matmuls, tile by tile (pipelined)
                for jt in sets[g]:
                    t = 32 * g + 4 * (jt // 8) + (jt % 8) - 4 * g
                    nc.vector.tensor_scalar(
                        out=fcY[:, jt * 128 : (jt + 1) * 128],
                        in0=fcY[:, jt * 128 : (jt + 1) * 128],
                        scalar1=dis_cols[:, t : t + 1], scalar2=None,
                        op0=mybir.AluOpType.mult,
                    )
                for jt in sets[g]:
                    for h in range(2):
                        nc.tensor.matmul(
                            oT[h][:], fcY[:, jt * 128 : (jt + 1) * 128],
                            ATC[:, jt * 1024 + h * HALF : jt * 1024 + (h + 1) * HALF],
                            start=(jt == sets[0][0]), stop=(jt == sets[1][-1]),
                        )
                if g == 0:
                    # idle gap while AllGather-2 flies: local dis_row = 1/s and
                    # the dis_i partition-broadcast for the epilogue
                    for ph in range(2):
                        nc.vector.reciprocal(
                            dis_row[0:1, ph * HALF : (ph + 1) * HALF], srows[ph][:]
                        )
                        nc.tensor.matmul(
                            bc_ps[ph][:], ones_row[:],
                            dis_row[0:1, ph * HALF : (ph + 1) * HALF],
                            start=True, stop=True,
                        )

            # ---- epilogue: scale by dis_i, DMA out ----
            for h in range(2):
                dbc[h] = epip.tile([128, HALF], F32, tag="dbc", name=f"dbc{h}")
                nc.vector.tensor_copy(dbc[h][:], bc_ps[h][:])
                osb = epip.tile([128, HALF], F32, tag="osb")
                nc.vector.tensor_tensor(
                    out=osb[:], in0=oT[h][:], in1=dbc[h][:], op=mybir.AluOpType.mult,
                )
                nc.sync.dma_start(out=outT[:, h * HALF : (h + 1) * HALF], in_=osb[:])

    nc.compile()
    return nc


def kernel(values, adjacency, W, b):
    from concourse.bass_utils import run_bass_kernel_spmd

    if "nc" not in _CACHE:
        _CACHE["nc"] = _build()
    nc = _CACHE["nc"]

    values = np.asarray(values, dtype=np.float32)
    adjacency = np.asarray(adjacency, dtype=np.float32)
    W = np.asarray(W, dtype=np.float32)
    b = np.asarray(b, dtype=np.float32)

    vt = np.ascontiguousarray(values.T)                  # [D, N]
    bb = np.ascontiguousarray(np.tile(b[None, :], (128, 1)))
    ident = np.eye(128, dtype=np.float32)

    in_maps = [
        {
            "at": np.ascontiguousarray(adjacency[k * ROWS : (k + 1) * ROWS].T),
            "vt": vt, "w": W, "bb": bb, "ident": ident,
        }
        for k in range(N_CORES)
    ]
    trace = bool(int(os.environ.get("GCN_TRACE", "0")))
    res = run_bass_kernel_spmd(nc, in_maps, list(range(N_CORES)), trace=trace)
    if trace and res.exec_time_ns is not None:
        print(f"HW exec time: {res.exec_time_ns} ns")
        _CACHE["exec_time_ns"] = res.exec_time_ns
    out = np.concatenate(
        [res.results[k]["outT"].T for k in range(N_CORES)], axis=0
    ).astype(np.float32)
    return out


# revision 20
# speedup vs baseline: 1.6396x; 1.0177x over previous
"""GCN layer on 8 Trainium2 NeuronCores.

out = D^-1/2 A D^-1/2 (values @ W + b),  A: [8192, 8192] f32 dense.

Strategy (row-parallel, host-transposed slabs, split-gather overlap):
- Core k owns output rows Rk = [1024k, 1024(k+1)). Host passes the slab
  pre-transposed: at = A[Rk, :].T  -> [8192 j, 1024 i], so tiles DMA with
  the contraction dim j on partitions (no on-device PE transposes).
- Stream at in two i-phases (cols 0:512, then 512:1024). Each 512KB stage
  DMA carries 2 j-tiles; DVE casts fp32->bf16 into a resident 16MB cache
  ATC [j-part, jt*1024 + i]. Row sums d (ones^T @ tile) accumulate in
  two ping-pong PSUM banks per phase, overlapping the stream.
- After phase A: dis_a = Rsqrt(d_a) locally -> AllGather-1 runs while
  phase B streams (values^T + fc also stream/compute in phase B).
  After phase B: AllGather-2 runs while the 32 S1 j-tile main matmuls
  execute. A tiny warm-up AllGather at kernel start absorbs the CC
  engine's first-collective init cost.
- dis distribution: one contiguous DMA [32,128] + one PE transpose.
- Y = fc * dis_j in place (bf16); main matmul out^T[o,i] += Y_jt^T @
  ATC_jt; epilogue scales by dis_i (partition-broadcast via K=1 matmul,
  precomputed per half as soon as local dis is ready); host transposes
  out^T back.
"""
import os
import numpy as np

N, D, OUT = 8192, 128, 128
N_CORES = 8
ROWS = N // N_CORES          # 1024 rows of A per core
NJT = N // 128               # 64 j-tiles
NST = 32                     # stages per phase (2 j-tiles each)
HALF = 512                   # i-split width per phase

_CACHE = {}


def _build():
    import concourse.bacc as bacc
    import concourse.mybir as mybir
    import concourse.tile as tile

    F32, BF16 = mybir.dt.float32, mybir.dt.bfloat16
    nc = bacc.Bacc(None, target_bir_lowering=False, num_devices=N_CORES)

    at_in = nc.declare_dram_parameter("at", [N, ROWS], F32, isOutput=False)
    vt_in = nc.declare_dram_parameter("vt", [D, N], F32, isOutput=False)
    w_in = nc.declare_dram_parameter("w", [D, OUT], F32, isOutput=False)
    bb_in = nc.declare_dram_parameter("bb", [128, OUT], F32, isOutput=False)
    id_in = nc.declare_dram_parameter("ident", [128, 128], F32, isOutput=False)
    outT = nc.declare_dram_parameter("outT", [OUT, ROWS], F32, isOutput=True)

    with tile.TileContext(nc) as tc:
        with (
            tc.tile_pool(name="const", bufs=1) as constp,
            tc.tile_pool(name="stage", bufs=8) as stage,
            tc.tile_pool(name="epi", bufs=2) as epip,
            tc.tile_pool(name="vtb", bufs=2) as vtbp,
            tc.tile_pool(name="small", bufs=1) as small,
            tc.tile_pool(name="ps", bufs=2, space="PSUM") as ps,
            tc.tile_pool(name="po", bufs=1, space="PSUM") as po,
            tc.tile_pool(name="pd", bufs=1, space="PSUM") as pd,
            tc.tile_pool(name="dram", bufs=1, space="DRAM") as dram,
        ):
            # ---- constants ----
            ident = constp.tile([128, 128], F32)
            nc.sync.dma_start(out=ident[:], in_=id_in[:])
            w_sb = constp.tile([D, OUT], F32)
            nc.sync.dma_start(out=w_sb[:], in_=w_in[:])
            w_bf = constp.tile([D, OUT], BF16)
            nc.vector.tensor_copy(w_bf[:], w_sb[:])
            bb_sb = constp.tile([128, OUT], F32)
            nc.sync.dma_start(out=bb_sb[:], in_=bb_in[:])
            ones_bf = constp.tile([128, 1], BF16)
            nc.vector.memset(ones_bf[:], 1.0)
            ones_row = constp.tile([1, 128], F32)
            nc.vector.memset(ones_row[:], 1.0)
            Z = constp.tile([128, 128], F32)
            nc.vector.memset(Z[:], 0.0)

            # warm-up collective: absorbs CC mesh-init + launch skew early,
            # while the stream is DMA-bound and the CC engine is idle.
            wu_loc = dram.tile([8], F32, name="wuloc")
            wu_full = dram.tile([8 * N_CORES], F32, addr_space="Shared", name="wufull")
            nc.sync.dma_start(out=wu_loc[:], in_=ones_row[0:1, 0:8])
            nc.gpsimd.collective_compute(
                "AllGather", mybir.AluOpType.bypass,
                replica_groups=[list(range(N_CORES))],
                ins=[wu_loc[:].opt()], outs=[wu_full[:].opt()],
            )

            # ---- big persistent buffers ----
            ATC = constp.tile([128, NJT * 1024], BF16)   # 16MB transposed A (bf16)
            fcY = constp.tile([128, NJT * 128], BF16)    # 2MB fc_sc, then Y in place
            dis_cols = constp.tile([128, 64], F32)       # dis_j per tile column
            dis_row = constp.tile([1, ROWS], F32)        # local dis_i row

            ATC3 = ATC[:].rearrange("p (j i) -> p j i", j=NJT)

            # ---- degree accumulators (one PSUM bank per phase) ----
            d_ps = [pd.tile([1, HALF], F32, tag=f"d{x}", name=f"dps{x}") for x in range(2)]
            oT = [po.tile([128, HALF], F32, tag=f"o{h}", name=f"oT{h}") for h in range(2)]
            dbc = [None, None]
            dis_loc = [None, None]
            dis_full = [None, None]
            drows = [None, None]

            # ---- stream phases: ph 0 = i cols [0,512), ph 1 = [512,1024) ----
            for ph in range(2):
                if ph == 1:
                    # fc = values @ W + b: streamed in phase B where DMA has slack
                    for c in range(8):
                        vstg = stage.tile([128, 1024], F32, tag="stg")
                        nc.sync.dma_start(
                            out=vstg[:], in_=vt_in[:, c * 1024 : (c + 1) * 1024]
                        )
                        vb = vtbp.tile([128, 1024], BF16, tag="vtb")
                        nc.vector.tensor_copy(vb[:], vstg[:])
                        for m in range(8):
                            nt = c * 8 + m
                            fc_ps = ps.tile([128, OUT], F32, tag="fc")
                            nc.tensor.matmul(
                                fc_ps[:], vb[:, m * 128 : (m + 1) * 128], w_bf[:],
                                start=True, stop=True,
                            )
                            nc.vector.tensor_tensor(
                                out=fcY[:, nt * 128 : (nt + 1) * 128],
                                in0=fc_ps[:], in1=bb_sb[:], op=mybir.AluOpType.add,
                            )
                for s in range(NST):
                    st = stage.tile([128, 1024], F32, tag="stg")
                    nc.sync.dma_start(
                        out=st[:].rearrange("p (q c) -> p q c", q=2),
                        in_=at_in[
                            s * 256 : (s + 1) * 256, ph * HALF : (ph + 1) * HALF
                        ].rearrange("(q p) c -> p q c", p=128),
                    )
                    nc.vector.tensor_copy(
                        ATC3[:, 2 * s : 2 * s + 2, ph * HALF : (ph + 1) * HALF],
                        st[:].rearrange("p (q c) -> p q c", q=2),
                    )
                    for q in range(2):
                        jt = 2 * s + q
                        nc.tensor.matmul(
                            d_ps[ph][:], ones_bf[:],
                            ATC[:, jt * 1024 + ph * HALF : jt * 1024 + (ph + 1) * HALF],
                            start=(s == 0 and q == 0), stop=(s == NST - 1 and q == 1),
                        )
                # gather RAW degree; sqrt/reciprocal run post-gather on wide
                # layouts (the 1e-8 guard is dropped: d ~ 4096 >> 0, and the
                # shift it causes is ~1.6e-10 relative)
                drow = small.tile([1, HALF], F32, tag=f"drow{ph}")
                nc.vector.tensor_copy(drow[:], d_ps[ph][:])
                dis_loc[ph] = dram.tile([HALF], F32, name=f"disloc{ph}")
                dis_full[ph] = dram.tile(
                    [HALF * N_CORES], F32, addr_space="Shared", name=f"disfull{ph}"
                )
                nc.sync.dma_start(out=dis_loc[ph][:], in_=drow[:])
                nc.gpsimd.collective_compute(
                    "AllGather", mybir.AluOpType.bypass,
                    replica_groups=[list(range(N_CORES))],
                    ins=[dis_loc[ph][:].opt()], outs=[dis_full[ph][:].opt()],
                )
                drows[ph] = drow

            # ---- per-gather: distribute dis, scale Y, run main matmuls ----
            # tiles whose dis arrives in gather g: jt%8 in [4g, 4g+4)
            sets = [
                [jt for jt in range(NJT) if (jt % 8) // 4 == g] for g in range(2)
            ]
            for g in range(2):
                # dis_full[g] -> Z rows -> PE transpose -> dis_cols[:, 32g:32g+32]
                nc.sync.dma_start(
                    out=Z[0:32, :],
                    in_=dis_full[g][:].rearrange("(t p) -> t p", p=128),
                )
                nc.scalar.activation(
                    Z[0:32, :], Z[0:32, :], mybir.ActivationFunctionType.Sqrt
                )
                zt_ps = ps.tile([128, 128], F32, tag="fc")
                nc.tensor.matmul(zt_ps[:], Z[:], ident[:], is_transpose=True,
                                 start=True, stop=True)
                nc.vector.reciprocal(
                    dis_cols[:, 32 * g : 32 * g + 32], zt_ps[:, 0:32]
                )
                # Y = fc * dis_j, then main matmuls, tile by tile (pipelined)
                for jt in sets[g]:
                    t = 32 * g + 4 * (jt // 8) + (jt % 8) - 4 * g
                    nc.vector.tensor_scalar(
                        out=fcY[:, jt * 128 : (jt + 1) * 128],
                        in0=fcY[:, jt * 128 : (jt + 1) * 128],
                        scalar1=dis_cols[:, t : t + 1], scalar2=None,
                        op0=mybir.AluOpType.mult,
                    )
                for h in range(2):
                    for jt in sets[g]:
                        nc.tensor.matmul(
                            oT[h][:], fcY[:, jt * 128 : (jt + 1) * 128],
                            ATC[:, jt * 1024 + h * HALF : jt * 1024 + (h + 1) * HALF],
                            start=(g == 0 and jt == sets[0][0]),
                            stop=(g == 1 and jt == sets[1][-1]),
                        )
                if g == 0:
                    # idle gap while AllGather-2 flies: local dis_row = 1/sqrt(d)
                    # and the dis_i partition-broadcast for the epilogue
                    for ph in range(2):
                        srow = small.tile([1, HALF], F32, tag=f"srow{ph}")
                        nc.scalar.activation(
                            srow[:], drows[ph][:], mybir.ActivationFunctionType.Sqrt
                        )
                        nc.vector.reciprocal(
                            dis_row[0:1, ph * HALF : (ph + 1) * HALF], srow[:]
                        )
                        bc_ps = ps.tile([128, HALF], F32, tag="bc")
                        nc.tensor.matmul(
                            bc_ps[:], ones_row[:],
                            dis_row[0:1, ph * HALF : (ph + 1) * HALF],
                            start=True, stop=True,
                        )
                        dbc[ph] = epip.tile([128, HALF], F32, tag="dbc", name=f"dbc{ph}")
                        nc.vector.tensor_copy(dbc[ph][:], bc_ps[:])

            # ---- epilogue: scale by dis_i, DMA out (h=0 overlaps h=1 matmuls) ----
            for h in range(2):
                osb = epip.tile([128, HALF], F32, tag="osb")
                nc.vector.tensor_tensor(
                    out=osb[:], in0=oT[h][:], in1=dbc[h][:], op=mybir.AluOpType.mult,
                )
                nc.sync.dma_start(out=outT[:, h * HALF : (h + 1) * HALF], in_=osb[:])

    nc.compile()
    return nc


def kernel(values, adjacency, W, b):
    from concourse.bass_utils import run_bass_kernel_spmd

    if "nc" not in _CACHE:
        _CACHE["nc"] = _build()
    nc = _CACHE["nc"]

    values = np.asarray(values, dtype=np.float32)
    adjacency = np.asarray(adjacency, dtype=np.float32)
    W = np.asarray(W, dtype=np.float32)
    b = np.asarray(b, dtype=np.float32)

    vt = np.ascontiguousarray(values.T)                  # [D, N]
    bb = np.ascontiguousarray(np.tile(b[None, :], (128, 1)))
    ident = np.eye(128, dtype=np.float32)

    in_maps = [
        {
            "at": np.ascontiguousarray(adjacency[k * ROWS : (k + 1) * ROWS].T),
            "vt": vt, "w": W, "bb": bb, "ident": ident,
        }
        for k in range(N_CORES)
    ]
    trace = bool(int(os.environ.get("GCN_TRACE", "0")))
    res = run_bass_kernel_spmd(nc, in_maps, list(range(N_CORES)), trace=trace)
    if trace and res.exec_time_ns is not None:
        print(f"HW exec time: {res.exec_time_ns} ns")
        _CACHE["exec_time_ns"] = res.exec_time_ns
    out = np.concatenate(
        [res.results[k]["outT"].T for k in range(N_CORES)], axis=0
    ).astype(np.float32)
    return out
